# revision 20
# baseline (speedup 1.0000x reference)
"""Trainium2 Bass kernel v4 for AttentionSequencePoolingLayer.

Keys are DMA'd with an inline fp32->bf16 cast on the SWDGE (gpsimd)
path -- 16 DMA engines at ~32ns/KB descriptor vs 10 at 57ns on HWDGE --
and no separate cast op. Per-item fused L1 weights w1q (80 cols) make
layer 1 one matmul per item; the per-item constant C is broadcast into
PSUM by a cheap identity matmul per pair (keeps ACT instruction count
low). relu1/relu2 run per-quad on ACT (800 cols each), kt evacuations
run on DVE as int32 (half the elements), masking is one fused
(score+b3)*mask DVE op, and w1q is built per-sg with two DVE ops.

Sharding: pure data parallel, batch split across 8 cores (256 each).
"""

from contextlib import ExitStack

import numpy as np

import concourse.bass as bass
import concourse.bacc as bacc
import concourse.tile as tile
from concourse import mybir
from concourse.bass_utils import run_bass_kernel_spmd
from concourse.masks import make_identity

B, T, E = 2048, 200, 128
H1, H2 = 80, 40
N_CORES = 8
B_CORE = B // N_CORES   # 256
SG = 16                 # items per supergroup (one keys DMA)
TH = T // 2             # 100 (t-pairs on partitions)
NP = B_CORE // 2        # 128 pairs
NSG = B_CORE // SG      # 16

F32 = mybir.dt.float32
F32R = mybir.dt.float32r
BF16 = mybir.dt.bfloat16
I32 = mybir.dt.int32
U8 = mybir.dt.uint8
AF = mybir.ActivationFunctionType
OP = mybir.AluOpType


def build(b_core=B_CORE):
    nc = bacc.Bacc("TRN2", target_bir_lowering=False, debug=False,
                   num_devices=N_CORES)
    q_d = nc.dram_tensor("query", [b_core, 1, E], F32, kind="ExternalInput")
    k_d = nc.dram_tensor("keys", [b_core, T, E], F32, kind="ExternalInput")
    m_d = nc.dram_tensor("key_masks", [b_core, 1, T], U8, kind="ExternalInput")
    w1_d = nc.dram_tensor("W1", [4 * E, H1], F32, kind="ExternalInput")
    b1_d = nc.dram_tensor("b1", [H1], F32, kind="ExternalInput")
    w2_d = nc.dram_tensor("W2", [H1, H2], F32, kind="ExternalInput")
    b2_d = nc.dram_tensor("b2", [H2], F32, kind="ExternalInput")
    w3_d = nc.dram_tensor("W3", [H2, 1], F32, kind="ExternalInput")
    b3_d = nc.dram_tensor("b3", [1], F32, kind="ExternalInput")
    out_d = nc.dram_tensor("out", [b_core, 1, E], F32, kind="ExternalOutput")

    with tile.TileContext(nc) as tc:
        _body(tc, nc, q_d, k_d, m_d, w1_d, b1_d, w2_d, b2_d, w3_d, b3_d,
              out_d, b_core)
    nc.compile()
    return nc


def _body(tc, nc, q_d, k_d, m_d, w1_d, b1_d, w2_d, b2_d, w3_d, b3_d, out_d,
          b_core):
    ctx = ExitStack()
    with ctx:
        consts = ctx.enter_context(tc.tile_pool(name="consts", bufs=1))
        prep = ctx.enter_context(tc.tile_pool(name="prep", bufs=2))
        ktp_ps = ctx.enter_context(tc.tile_pool(name="ktp_ps", bufs=1,
                                                space="PSUM"))
        h1_ps = ctx.enter_context(tc.tile_pool(name="h1_ps", bufs=1,
                                               space="PSUM"))
        h2_ps = ctx.enter_context(tc.tile_pool(name="h2_ps", bufs=1,
                                               space="PSUM"))
        sml_ps = ctx.enter_context(tc.tile_pool(name="sml_ps", bufs=2,
                                                space="PSUM"))

        identf = consts.tile([128, 128], F32)
        make_identity(nc, identf)
        identb = consts.tile([128, 128], BF16)
        nc.vector.tensor_copy(out=identb, in_=identf)

        # ---- weights ----
        w1s = consts.tile([128, 4, H1], F32)
        nc.sync.dma_start(out=w1s, in_=w1_d.rearrange("(f p) c -> p f c", p=128))
        w1ac = consts.tile([128, H1], F32R)
        nc.vector.tensor_tensor(out=w1ac, in0=w1s[:, 0, :], in1=w1s[:, 2, :],
                                op=OP.add)
        w1db = consts.tile([128, H1], BF16)
        nc.vector.tensor_copy(out=w1db, in_=w1s[:, 3, :])
        w1bcb = consts.tile([128, H1], BF16)
        nc.vector.tensor_tensor(out=w1bcb, in0=w1s[:, 1, :], in1=w1s[:, 2, :],
                                op=OP.subtract)
        w2f = consts.tile([H1, H2], F32)
        nc.sync.dma_start(out=w2f, in_=w2_d.ap())
        w2b = consts.tile([128, H2], BF16)
        nc.vector.memset(w2b, 0.0)
        nc.vector.tensor_copy(out=w2b[0:H1, :], in_=w2f)
        w3f = consts.tile([H2, 1], F32)
        nc.sync.dma_start(out=w3f, in_=w3_d.ap())
        w3p = consts.tile([H2, 1], BF16)
        nc.vector.tensor_copy(out=w3p, in_=w3f)
        b1 = consts.tile([H1, 1], F32)
        nc.sync.dma_start(
            out=b1, in_=bass.AP(tensor=b1_d.ap().tensor, offset=0,
                                ap=[[1, H1], [1, 1]]))
        b2 = consts.tile([H2, 1], F32)
        nc.sync.dma_start(
            out=b2, in_=bass.AP(tensor=b2_d.ap().tensor, offset=0,
                                ap=[[1, H2], [1, 1]]))
        b3bc = consts.tile([TH, 1], F32)
        nc.sync.dma_start(
            out=b3bc, in_=bass.AP(tensor=b3_d.ap().tensor, offset=0,
                                  ap=[[0, TH], [1, 1]]))

        # ---- Q.T (E on partitions, batch on free) ----
        qt = consts.tile([128, b_core], F32R)
        q_flat = q_d.rearrange("b 1 e -> b e")
        for i in range(0, b_core, 128):
            cnt = min(128, b_core - i)
            qnat = prep.tile([128, E], F32, tag="qnat")
            nc.sync.dma_start(out=qnat[:cnt, :], in_=q_flat[i:i + cnt, :])
            qps = sml_ps.tile([128, 128], F32, tag="sml", name="qps")
            nc.tensor.transpose(qps[:, :cnt], qnat[:cnt, :], identf[:cnt, :cnt])
            nc.vector.tensor_copy(out=qt[:, i:i + cnt], in_=qps[:, :cnt])

        # ---- C = (W1a+W1c).T @ Q.T + b1 -> csbb [128, b_core] bf16 ----
        cps = sml_ps.tile([H1, b_core], F32, tag="sml", name="cps")
        nc.tensor.matmul(cps, lhsT=w1ac, rhs=qt, start=True, stop=True)
        csbb = consts.tile([128, b_core], BF16)
        nc.vector.memset(csbb, 0.0)
        nc.scalar.activation(out=csbb[0:H1, :], in_=cps, func=AF.Identity,
                             bias=b1)

        # masks prepared later (mask_prep), after prologue casts
        mtE = consts.tile([TH, b_core], F32)
        mtO = consts.tile([TH, b_core], F32)

        def mask_prep():
            m_flat = m_d.rearrange("b 1 t -> b t")
            for i in range(0, b_core, 128):
                cnt = min(128, b_core - i)
                mu8 = prep.tile([128, T], U8, tag="mu8")
                nc.sync.dma_start(out=mu8[:cnt, :], in_=m_flat[i:i + cnt, :])
                mf = prep.tile([128, T], F32, tag="mf")
                nc.vector.tensor_copy(out=mf[:cnt, :], in_=mu8[:cnt, :])
                mfv = mf.rearrange("p (t two) -> p two t", two=2)
                for par, mt in ((0, mtE), (1, mtO)):
                    mps = sml_ps.tile([128, 128], F32, tag="sml", name="mps")
                    nc.tensor.transpose(mps[:TH, :cnt], mfv[:cnt, par, :],
                                        identf[:cnt, :cnt])
                    nc.vector.tensor_copy(out=mt[:, i:i + cnt],
                                          in_=mps[:TH, :cnt])

        qtb = consts.tile([128, b_core], BF16)
        nc.vector.tensor_copy(out=qtb, in_=qt.bitcast(F32))

        # pooled output, transposed: (E, batch)
        poolt_sb = consts.tile([128, b_core], F32)

        # ---- main pipeline pools ----
        kbp = ctx.enter_context(tc.tile_pool(name="kbp", bufs=7))
        w1qp = ctx.enter_context(tc.tile_pool(name="w1qp", bufs=3))
        ktq = ctx.enter_context(tc.tile_pool(name="ktq", bufs=3))
        h1qp = ctx.enter_context(tc.tile_pool(name="h1qp", bufs=3))
        h2qp = ctx.enter_context(tc.tile_pool(name="h2qp", bufs=3))
        smp = ctx.enter_context(tc.tile_pool(name="smp", bufs=2))

        # psum tiles: bank-safe layouts (no matmul write crosses a 2KB bank)
        ktpA = ktp_ps.tile([128, 2, 2, TH * 2], BF16, name="ktpA")  # 1 bank
        ktpB = ktp_ps.tile([128, 2, 2, TH * 2], BF16, name="ktpB")  # 1 bank
        h1ps = h1_ps.tile([128, 2, 512], F32, name="h1ps")  # 2 banks, padded
        h2ps = h2_ps.tile([H2, 2, 512], F32, name="h2ps")   # 2 banks, padded

        kE = T * E  # element stride between items in keys dram
        k_t = k_d.ap().tensor

        state = {}

        def dma_kb(sg, nsplit=1):
            if not (0 <= sg < NSG):
                return
            kb = kbp.tile([TH, SG, 2, E], BF16, tag="kb", name="kb")
            kbv = kb.rearrange("p s a e -> p s (a e)")
            step = SG // nsplit
            for c in range(nsplit):
                nc.gpsimd.dma_start(
                    out=kbv[:, c * step:(c + 1) * step, :],
                    in_=bass.AP(tensor=k_t,
                                offset=(sg * SG + c * step) * kE,
                                ap=[[2 * E, TH], [kE, step], [1, 2 * E]]))
            state[("kb", sg)] = kb

        def w1q_slab(sg):
            # w1q[:, i, 0:80] = (W1d * q_i) + (W1b - W1c); cols 80:128 zero
            # so the L1 lhsT is 128-wide (FWL). Two DVE ops + a pad memset.
            if not (0 <= sg < NSG):
                return
            b0 = sg * SG
            tmp = w1qp.tile([128, SG, H1], BF16, tag="w1qtmp", name="tmp",
                            bufs=2)
            w1d3 = bass.AP(tensor=w1db.tensor, offset=w1db.offset,
                           ap=[w1db.ap[0], [0, SG], w1db.ap[1]])
            qsl = qtb[:, b0:b0 + SG]
            qt3 = bass.AP(tensor=qsl.tensor, offset=qsl.offset,
                          ap=[qsl.ap[0], qsl.ap[1], [0, H1]])
            nc.gpsimd.tensor_tensor(out=tmp, in0=w1d3, in1=qt3, op=OP.mult)
            w1q = w1qp.tile([128, SG, 128], BF16, tag="w1q", name="w1q")
            if sg < 3:
                # pads zeroed once per physical buffer (bufs=3, rotation)
                nc.vector.memset(w1q[:, :, H1:128], 0.0)
            w1bc3 = bass.AP(tensor=w1bcb.tensor, offset=w1bcb.offset,
                            ap=[w1bcb.ap[0], [0, SG], w1bcb.ap[1]])
            nc.gpsimd.tensor_tensor(out=w1q[:, :, 0:H1], in0=tmp, in1=w1bc3,
                                    op=OP.add)
            state[("w1q", sg)] = w1q

        def stage_A(P):  # transposes of pair P -> ktp quad tile, half P%2
            sg, loc = P // 8, P % 8
            kb = state[("kb", sg)]
            ktp = ktpA if (P // 2) % 2 == 0 else ktpB
            half = P % 2
            ops = []
            for j in range(2):
                item = 2 * loc + j
                for par in range(2):
                    ops.append(lambda j=j, item=item, par=par: nc.tensor.transpose(
                        ktp[:, half, j, par * TH:(par + 1) * TH],
                        kb[:, item, par, :], identb[:TH, :TH]))
            return ops

        def ktevac(P):  # evacuate one quad's ktp tile -> kt sbuf (DVE, i32)
            q = (P - 1) // 2
            ktp = ktpA if q % 2 == 0 else ktpB
            kt = ktq.tile([128, 2, 2, 2 * TH], BF16, tag="kt", name="kt")
            ktv = kt.rearrange("p a b t -> p (a b t)").bitcast(I32)
            ktpv = ktp.rearrange("p a b t -> p (a b t)").bitcast(I32)
            nc.vector.tensor_copy(out=ktv, in_=ktpv)
            state[("kt", q)] = kt

        def stage_B(P):  # L1 for pair P: C broadcast + matmul per item
            q, half = P // 2, P % 2
            kt = state[("kt", q)]
            sg = P // 8
            w1q = state[("w1q", sg)]
            csl = csbb[:, 2 * P:2 * P + 2]
            crhs = bass.AP(tensor=csl.tensor, offset=csl.offset,
                           ap=[csl.ap[0], csl.ap[1], [0, 200]])
            ops = [lambda: nc.tensor.matmul(
                h1ps[:, half, 0:400], lhsT=identb, rhs=crhs,
                start=True, stop=False)]
            for j in range(2):
                item = (P % 8) * 2 + j
                ops.append(lambda j=j, item=item: nc.tensor.matmul(
                    h1ps[:, half, j * 200:(j + 1) * 200],
                    lhsT=w1q[:, item, :], rhs=kt[:, half, j, :],
                    start=False, stop=(j == 1)))
            return ops

        def relu1(q):  # quad q: h1 = relu(h1ps), split ACT/DVE halves
            h1q = h1qp.tile([128, 2, 512], BF16, tag="h1q", name="h1q")
            nc.scalar.activation(out=h1q[:, :, 0:200], in_=h1ps[:, :, 0:200],
                                 func=AF.Relu)
            nc.vector.tensor_scalar(
                out=h1q[:, :, 200:400], in0=h1ps[:, :, 200:400],
                scalar1=0.0, scalar2=None, op0=OP.max)
            state[("h1q", q)] = h1q

        def stage_D(P):  # L2 for pair P (K=80)
            half = P % 2
            h1q = state[("h1q", P // 2)]
            return [lambda: nc.tensor.matmul(
                h2ps[:, half, 0:400], lhsT=w2b, rhs=h1q[:, half, 0:400],
                start=True, stop=True)]

        def relu2(q):  # quad q: h2 = relu(h2ps + b2) on ACT
            h2q = h2qp.tile([H2, 2, 512], BF16, tag="h2q", name="h2q")
            nc.scalar.activation(out=h2q[:, :, 0:400], in_=h2ps[:, :, 0:400],
                                 func=AF.Relu, bias=b2)
            state[("h2q", q)] = h2q

        def stage_F(P):  # score minis for pair P
            q, half = P // 2, P % 2
            h2q = state[("h2q", q)]
            sml = state[("sml", P // 8)]
            ops = []
            for j in range(2):
                li = (P % 8) * 2 + j
                for par in range(2):
                    ops.append(lambda j=j, li=li, par=par: nc.tensor.matmul(
                        sml[:, 16 * par + li:16 * par + li + 1],
                        lhsT=h2q[:, half, j * 200 + par * TH:
                                 j * 200 + par * TH + 128],
                        rhs=w3p, start=True, stop=True))
            return ops

        def alloc_sml(sg):
            sml = sml_ps.tile([128, 128], F32, tag="sml", name="sml")
            state[("sml", sg)] = sml

        def do_mask(sg, hf):  # sm = (score + b3) * mask, fused per parity
            b0 = sg * SG + 8 * hf
            sml = state[("sml", sg)]
            if hf == 0:
                sm = smp.tile([TH, 2, SG], BF16, tag="sm", name="sm")
                state[("sm", sg)] = sm
            sm = state[("sm", sg)]
            s = slice(8 * hf, 8 * hf + 8)
            for par in range(2):
                mt = mtE if par == 0 else mtO
                nc.vector.scalar_tensor_tensor(
                    out=sm[:, par, s],
                    in0=sml[0:TH, 16 * par + 8 * hf:16 * par + 8 * hf + 8],
                    scalar=b3bc, in1=mt[:, b0:b0 + 8],
                    op0=OP.add, op1=OP.mult)

        def pool_chunk(sg, loc):  # pool matmuls for items 2*loc, 2*loc+1
            kb = state[("kb", sg)]
            sml = state[("sml", sg)]
            sm = state[("sm", sg)]
            ops = []
            for j in range(2):
                item = 2 * loc + j
                c = 32 + item
                ops.append(lambda item=item, c=c: (
                    nc.tensor.matmul(sml[:, c:c + 1], lhsT=kb[:, item, 0, :],
                                     rhs=sm[:, 0, item:item + 1],
                                     start=True, stop=False),
                    nc.tensor.matmul(sml[:, c:c + 1], lhsT=kb[:, item, 1, :],
                                     rhs=sm[:, 1, item:item + 1],
                                     start=False, stop=True)))
            return ops

        def pool_evac(sg):
            b0 = sg * SG
            sml = state[("sml", sg)]
            nc.vector.tensor_copy(out=poolt_sb[:, b0:b0 + SG],
                                  in_=sml[:, 32:32 + SG])

        # ---- prologue ----
        dma_kb(0, nsplit=4)
        dma_kb(1)
        dma_kb(2)
        dma_kb(3)
        w1q_slab(0)
        mask_prep()
        alloc_sml(0)
        alloc_sml(1)

        # ---- main software-pipelined loop ----
        for P in range(NP + 24):
            t_ops = stage_A(P) if P < NP else []
            f_ops = stage_F(P - 8) if 8 <= P < NP + 8 else []
            b_ops = stage_B(P - 2) if 2 <= P < NP + 2 else []
            d_ops = stage_D(P - 4) if 4 <= P < NP + 4 else []
            p_ops = (pool_chunk((P - 16) // 8, (P - 16) % 8)
                     if 16 <= P < NP + 16 else [])
            for op in t_ops + b_ops + d_ops + f_ops + p_ops:
                op()

            if P % 2 == 1 and P < NP:
                ktevac(P)
            if P % 2 == 1 and 5 <= P < NP + 5:
                relu2((P - 5) // 2)
            if P % 2 == 1 and 3 <= P < NP + 3:
                relu1((P - 3) // 2)
            sg = P // 8
            r = P % 8
            if r == 0 and P < NP:
                dma_kb(sg + 4)
            elif r == 5:
                w1q_slab(sg + 1)
            if 8 <= P < NP + 8 and (P - 8) % 8 == 3:
                do_mask((P - 8) // 8, 0)
            if 8 <= P < NP + 8 and (P - 8) % 8 == 7:
                do_mask((P - 8) // 8, 1)
                if (P - 8) // 8 + 2 < NSG:
                    alloc_sml((P - 8) // 8 + 2)
            if 16 <= P < NP + 16 and (P - 16) % 8 == 7:
                pool_evac((P - 16) // 8)

        # ---- epilogue: transpose pooled back to (batch, E) and store ----
        out_flat = out_d.rearrange("b 1 e -> b e")
        for i in range(0, b_core, 128):
            cnt = min(128, b_core - i)
            ops = sml_ps.tile([128, 128], F32, tag="sml", name="ops")
            nc.tensor.transpose(ops[:cnt, :], poolt_sb[:, i:i + cnt], identf)
            onat = prep.tile([128, E], F32, tag="onat")
            nc.vector.tensor_copy(out=onat[:cnt, :], in_=ops[:cnt, :])
            nc.sync.dma_start(out=out_flat[i:i + cnt, :], in_=onat[:cnt, :])


_NC_CACHE = {}


def _get_nc(b_core=B_CORE):
    if b_core not in _NC_CACHE:
        _NC_CACHE[b_core] = build(b_core)
    return _NC_CACHE[b_core]


def kernel(query, keys, key_masks, W1, b1, W2, b2, W3, b3, _trace=False):
    query = np.ascontiguousarray(query, dtype=np.float32)
    keys = np.ascontiguousarray(keys, dtype=np.float32)
    masks_u8 = np.ascontiguousarray(key_masks).view(np.uint8)
    nc = _get_nc()
    in_maps = []
    for c in range(N_CORES):
        sl = slice(c * B_CORE, (c + 1) * B_CORE)
        in_maps.append({
            "query": query[sl],
            "keys": keys[sl],
            "key_masks": masks_u8[sl],
            "W1": np.asarray(W1, dtype=np.float32),
            "b1": np.asarray(b1, dtype=np.float32),
            "W2": np.asarray(W2, dtype=np.float32),
            "b2": np.asarray(b2, dtype=np.float32),
            "W3": np.asarray(W3, dtype=np.float32),
            "b3": np.asarray(b3, dtype=np.float32),
        })
    res = run_bass_kernel_spmd(nc, in_maps, list(range(N_CORES)), trace=_trace)
    out = np.concatenate([res.results[c]["out"] for c in range(N_CORES)], axis=0)
    if _trace:
        kernel.last_exec_time_ns = res.exec_time_ns
        kernel.last_results = res
    return out.astype(np.float32)


kernel.last_exec_time_ns = None
kernel.last_results = None


# revision 22
# speedup vs baseline: 1.0045x; 1.0045x over previous
"""Trainium2 Bass kernel v4 for AttentionSequencePoolingLayer.

Keys are DMA'd with an inline fp32->bf16 cast on the SWDGE (gpsimd)
path -- 16 DMA engines at ~32ns/KB descriptor vs 10 at 57ns on HWDGE --
and no separate cast op. Per-item fused L1 weights w1q (80 cols) make
layer 1 one matmul per item; the per-item constant C is broadcast into
PSUM by a cheap identity matmul per pair (keeps ACT instruction count
low). relu1/relu2 run per-quad on ACT (800 cols each), kt evacuations
run on DVE as int32 (half the elements), masking is one fused
(score+b3)*mask DVE op, and w1q is built per-sg with two DVE ops.

Sharding: pure data parallel, batch split across 8 cores (256 each).
"""

from contextlib import ExitStack

import numpy as np

import concourse.bass as bass
import concourse.bacc as bacc
import concourse.tile as tile
from concourse import mybir
from concourse.bass_utils import run_bass_kernel_spmd
from concourse.masks import make_identity

B, T, E = 2048, 200, 128
H1, H2 = 80, 40
N_CORES = 8
B_CORE = B // N_CORES   # 256
SG = 16                 # items per supergroup (one keys DMA)
TH = T // 2             # 100 (t-pairs on partitions)
NP = B_CORE // 2        # 128 pairs
NSG = B_CORE // SG      # 16

F32 = mybir.dt.float32
F32R = mybir.dt.float32r
BF16 = mybir.dt.bfloat16
I32 = mybir.dt.int32
U8 = mybir.dt.uint8
AF = mybir.ActivationFunctionType
OP = mybir.AluOpType


def build(b_core=B_CORE):
    nc = bacc.Bacc("TRN2", target_bir_lowering=False, debug=False,
                   num_devices=N_CORES)
    q_d = nc.dram_tensor("query", [b_core, 1, E], F32, kind="ExternalInput")
    k_d = nc.dram_tensor("keys", [b_core, T, E], F32, kind="ExternalInput")
    m_d = nc.dram_tensor("key_masks", [b_core, 1, T], U8, kind="ExternalInput")
    w1_d = nc.dram_tensor("W1", [4 * E, H1], F32, kind="ExternalInput")
    b1_d = nc.dram_tensor("b1", [H1], F32, kind="ExternalInput")
    w2_d = nc.dram_tensor("W2", [H1, H2], F32, kind="ExternalInput")
    b2_d = nc.dram_tensor("b2", [H2], F32, kind="ExternalInput")
    w3_d = nc.dram_tensor("W3", [H2, 1], F32, kind="ExternalInput")
    b3_d = nc.dram_tensor("b3", [1], F32, kind="ExternalInput")
    out_d = nc.dram_tensor("out", [b_core, 1, E], F32, kind="ExternalOutput")

    with tile.TileContext(nc) as tc:
        _body(tc, nc, q_d, k_d, m_d, w1_d, b1_d, w2_d, b2_d, w3_d, b3_d,
              out_d, b_core)
    nc.compile()
    return nc


def _body(tc, nc, q_d, k_d, m_d, w1_d, b1_d, w2_d, b2_d, w3_d, b3_d, out_d,
          b_core):
    ctx = ExitStack()
    with ctx:
        consts = ctx.enter_context(tc.tile_pool(name="consts", bufs=1))
        prep = ctx.enter_context(tc.tile_pool(name="prep", bufs=2))
        ktp_ps = ctx.enter_context(tc.tile_pool(name="ktp_ps", bufs=1,
                                                space="PSUM"))
        h1_ps = ctx.enter_context(tc.tile_pool(name="h1_ps", bufs=1,
                                               space="PSUM"))
        h2_ps = ctx.enter_context(tc.tile_pool(name="h2_ps", bufs=1,
                                               space="PSUM"))
        sml_ps = ctx.enter_context(tc.tile_pool(name="sml_ps", bufs=2,
                                                space="PSUM"))

        identf = consts.tile([128, 128], F32)
        make_identity(nc, identf)
        identb = consts.tile([128, 128], BF16)
        nc.vector.tensor_copy(out=identb, in_=identf)

        # ---- weights ----
        w1s = consts.tile([128, 4, H1], F32)
        nc.sync.dma_start(out=w1s, in_=w1_d.rearrange("(f p) c -> p f c", p=128))
        w1ac = consts.tile([128, H1], F32R)
        nc.vector.tensor_tensor(out=w1ac, in0=w1s[:, 0, :], in1=w1s[:, 2, :],
                                op=OP.add)
        w1db = consts.tile([128, H1], BF16)
        nc.vector.tensor_copy(out=w1db, in_=w1s[:, 3, :])
        w1bcb = consts.tile([128, H1], BF16)
        nc.vector.tensor_tensor(out=w1bcb, in0=w1s[:, 1, :], in1=w1s[:, 2, :],
                                op=OP.subtract)
        w2f = consts.tile([H1, H2], F32)
        nc.sync.dma_start(out=w2f, in_=w2_d.ap())
        w2b = consts.tile([128, H2], BF16)
        nc.vector.memset(w2b, 0.0)
        nc.vector.tensor_copy(out=w2b[0:H1, :], in_=w2f)
        w3f = consts.tile([H2, 1], F32)
        nc.sync.dma_start(out=w3f, in_=w3_d.ap())
        w3p = consts.tile([H2, 1], BF16)
        nc.vector.tensor_copy(out=w3p, in_=w3f)
        b1 = consts.tile([H1, 1], F32)
        nc.sync.dma_start(
            out=b1, in_=bass.AP(tensor=b1_d.ap().tensor, offset=0,
                                ap=[[1, H1], [1, 1]]))
        b2 = consts.tile([H2, 1], F32)
        nc.sync.dma_start(
            out=b2, in_=bass.AP(tensor=b2_d.ap().tensor, offset=0,
                                ap=[[1, H2], [1, 1]]))
        b3bc = consts.tile([TH, 1], F32)
        nc.sync.dma_start(
            out=b3bc, in_=bass.AP(tensor=b3_d.ap().tensor, offset=0,
                                  ap=[[0, TH], [1, 1]]))

        # ---- Q.T (E on partitions, batch on free) ----
        qt = consts.tile([128, b_core], F32R)
        q_flat = q_d.rearrange("b 1 e -> b e")
        for i in range(0, b_core, 128):
            cnt = min(128, b_core - i)
            qnat = prep.tile([128, E], F32, tag="qnat")
            nc.sync.dma_start(out=qnat[:cnt, :], in_=q_flat[i:i + cnt, :])
            qps = sml_ps.tile([128, 128], F32, tag="sml", name="qps")
            nc.tensor.transpose(qps[:, :cnt], qnat[:cnt, :], identf[:cnt, :cnt])
            nc.vector.tensor_copy(out=qt[:, i:i + cnt], in_=qps[:, :cnt])

        # ---- C = (W1a+W1c).T @ Q.T + b1 -> csbb [128, b_core] bf16 ----
        cps = sml_ps.tile([H1, b_core], F32, tag="sml", name="cps")
        nc.tensor.matmul(cps, lhsT=w1ac, rhs=qt, start=True, stop=True)
        csbb = consts.tile([128, b_core], BF16)
        nc.vector.memset(csbb, 0.0)
        nc.scalar.activation(out=csbb[0:H1, :], in_=cps, func=AF.Identity,
                             bias=b1)

        # masks prepared later (mask_prep), after prologue casts
        mtE = consts.tile([TH, b_core], F32)
        mtO = consts.tile([TH, b_core], F32)

        def mask_prep():
            m_flat = m_d.rearrange("b 1 t -> b t")
            for i in range(0, b_core, 128):
                cnt = min(128, b_core - i)
                mu8 = prep.tile([128, T], U8, tag="mu8")
                nc.sync.dma_start(out=mu8[:cnt, :], in_=m_flat[i:i + cnt, :])
                mf = prep.tile([128, T], F32, tag="mf")
                nc.vector.tensor_copy(out=mf[:cnt, :], in_=mu8[:cnt, :])
                mfv = mf.rearrange("p (t two) -> p two t", two=2)
                for par, mt in ((0, mtE), (1, mtO)):
                    mps = sml_ps.tile([128, 128], F32, tag="sml", name="mps")
                    nc.tensor.transpose(mps[:TH, :cnt], mfv[:cnt, par, :],
                                        identf[:cnt, :cnt])
                    nc.vector.tensor_copy(out=mt[:, i:i + cnt],
                                          in_=mps[:TH, :cnt])

        qtb = consts.tile([128, b_core], BF16)
        nc.vector.tensor_copy(out=qtb, in_=qt.bitcast(F32))

        # pooled output, transposed: (E, batch)
        poolt_sb = consts.tile([128, b_core], F32)

        # ---- main pipeline pools ----
        kbp = ctx.enter_context(tc.tile_pool(name="kbp", bufs=7))
        w1qp = ctx.enter_context(tc.tile_pool(name="w1qp", bufs=3))
        ktq = ctx.enter_context(tc.tile_pool(name="ktq", bufs=3))
        h1qp = ctx.enter_context(tc.tile_pool(name="h1qp", bufs=3))
        h2qp = ctx.enter_context(tc.tile_pool(name="h2qp", bufs=3))
        smp = ctx.enter_context(tc.tile_pool(name="smp", bufs=2))

        # psum tiles: bank-safe layouts (no matmul write crosses a 2KB bank)
        ktpA = ktp_ps.tile([128, 2, 2, TH * 2], BF16, name="ktpA")  # 1 bank
        ktpB = ktp_ps.tile([128, 2, 2, TH * 2], BF16, name="ktpB")  # 1 bank
        h1ps = h1_ps.tile([128, 2, 512], F32, name="h1ps")  # 2 banks, padded
        h2ps = h2_ps.tile([H2, 2, 512], F32, name="h2ps")   # 2 banks, padded

        kE = T * E  # element stride between items in keys dram
        k_t = k_d.ap().tensor

        state = {}

        def dma_kb(sg, nsplit=1):
            if not (0 <= sg < NSG):
                return
            kb = kbp.tile([TH, SG, 2, E], BF16, tag="kb", name="kb")
            kbv = kb.rearrange("p s a e -> p s (a e)")
            step = SG // nsplit
            for c in range(nsplit):
                nc.gpsimd.dma_start(
                    out=kbv[:, c * step:(c + 1) * step, :],
                    in_=bass.AP(tensor=k_t,
                                offset=(sg * SG + c * step) * kE,
                                ap=[[2 * E, TH], [kE, step], [1, 2 * E]]))
            state[("kb", sg)] = kb

        def w1q_slab(sg):
            # w1q[:, i, 0:80] = (W1d * q_i) + (W1b - W1c); cols 80:128 zero
            # so the L1 lhsT is 128-wide (FWL). Two DVE ops + a pad memset.
            if not (0 <= sg < NSG):
                return
            b0 = sg * SG
            tmp = w1qp.tile([128, SG, H1], BF16, tag="w1qtmp", name="tmp",
                            bufs=2)
            w1d3 = bass.AP(tensor=w1db.tensor, offset=w1db.offset,
                           ap=[w1db.ap[0], [0, SG], w1db.ap[1]])
            qsl = qtb[:, b0:b0 + SG]
            qt3 = bass.AP(tensor=qsl.tensor, offset=qsl.offset,
                          ap=[qsl.ap[0], qsl.ap[1], [0, H1]])
            nc.vector.tensor_tensor(out=tmp, in0=w1d3, in1=qt3, op=OP.mult)
            w1q = w1qp.tile([128, SG, 128], BF16, tag="w1q", name="w1q")
            if sg < 3:
                # pads zeroed once per physical buffer (bufs=3, rotation)
                nc.vector.memset(w1q[:, :, H1:128], 0.0)
            w1bc3 = bass.AP(tensor=w1bcb.tensor, offset=w1bcb.offset,
                            ap=[w1bcb.ap[0], [0, SG], w1bcb.ap[1]])
            nc.vector.tensor_tensor(out=w1q[:, :, 0:H1], in0=tmp, in1=w1bc3,
                                    op=OP.add)
            state[("w1q", sg)] = w1q

        def stage_A(P):  # transposes of pair P -> ktp quad tile, half P%2
            sg, loc = P // 8, P % 8
            kb = state[("kb", sg)]
            ktp = ktpA if (P // 2) % 2 == 0 else ktpB
            half = P % 2
            ops = []
            for j in range(2):
                item = 2 * loc + j
                for par in range(2):
                    ops.append(lambda j=j, item=item, par=par: nc.tensor.transpose(
                        ktp[:, half, j, par * TH:(par + 1) * TH],
                        kb[:, item, par, :], identb[:TH, :TH]))
            return ops

        def ktevac(P):  # evacuate one quad's ktp tile -> kt sbuf (DVE, i32)
            q = (P - 1) // 2
            ktp = ktpA if q % 2 == 0 else ktpB
            kt = ktq.tile([128, 2, 2, 2 * TH], BF16, tag="kt", name="kt")
            ktv = kt.rearrange("p a b t -> p (a b t)").bitcast(I32)
            ktpv = ktp.rearrange("p a b t -> p (a b t)").bitcast(I32)
            nc.vector.tensor_copy(out=ktv, in_=ktpv)
            state[("kt", q)] = kt

        def stage_B(P):  # L1 for pair P: C broadcast + matmul per item
            q, half = P // 2, P % 2
            kt = state[("kt", q)]
            sg = P // 8
            w1q = state[("w1q", sg)]
            csl = csbb[:, 2 * P:2 * P + 2]
            crhs = bass.AP(tensor=csl.tensor, offset=csl.offset,
                           ap=[csl.ap[0], csl.ap[1], [0, 200]])
            ops = [lambda: nc.tensor.matmul(
                h1ps[:, half, 0:400], lhsT=identb, rhs=crhs,
                start=True, stop=False)]
            for j in range(2):
                item = (P % 8) * 2 + j
                ops.append(lambda j=j, item=item: nc.tensor.matmul(
                    h1ps[:, half, j * 200:(j + 1) * 200],
                    lhsT=w1q[:, item, :], rhs=kt[:, half, j, :],
                    start=False, stop=(j == 1)))
            return ops

        def relu1(q):  # quad q: h1 = relu(h1ps), split ACT/DVE halves
            h1q = h1qp.tile([128, 2, 512], BF16, tag="h1q", name="h1q")
            nc.scalar.activation(out=h1q[:, :, 0:200], in_=h1ps[:, :, 0:200],
                                 func=AF.Relu)
            nc.vector.tensor_scalar(
                out=h1q[:, :, 200:400], in0=h1ps[:, :, 200:400],
                scalar1=0.0, scalar2=None, op0=OP.max)
            state[("h1q", q)] = h1q

        def stage_D(P):  # L2 for pair P (K=80)
            half = P % 2
            h1q = state[("h1q", P // 2)]
            return [lambda: nc.tensor.matmul(
                h2ps[:, half, 0:400], lhsT=w2b, rhs=h1q[:, half, 0:400],
                start=True, stop=True)]

        def relu2(q):  # quad q: h2 = relu(h2ps + b2) on ACT
            h2q = h2qp.tile([H2, 2, 512], BF16, tag="h2q", name="h2q")
            nc.scalar.activation(out=h2q[:, :, 0:400], in_=h2ps[:, :, 0:400],
                                 func=AF.Relu, bias=b2)
            state[("h2q", q)] = h2q

        def stage_F(P):  # score minis for pair P
            q, half = P // 2, P % 2
            h2q = state[("h2q", q)]
            sml = state[("sml", P // 8)]
            ops = []
            for j in range(2):
                li = (P % 8) * 2 + j
                for par in range(2):
                    ops.append(lambda j=j, li=li, par=par: nc.tensor.matmul(
                        sml[:, 16 * par + li:16 * par + li + 1],
                        lhsT=h2q[:, half, j * 200 + par * TH:
                                 j * 200 + par * TH + 128],
                        rhs=w3p, start=True, stop=True))
            return ops

        def alloc_sml(sg):
            sml = sml_ps.tile([128, 128], F32, tag="sml", name="sml")
            state[("sml", sg)] = sml

        def do_mask(sg, hf):  # sm = (score + b3) * mask, fused per parity
            b0 = sg * SG + 8 * hf
            sml = state[("sml", sg)]
            if hf == 0:
                sm = smp.tile([TH, 2, SG], BF16, tag="sm", name="sm")
                state[("sm", sg)] = sm
            sm = state[("sm", sg)]
            s = slice(8 * hf, 8 * hf + 8)
            for par in range(2):
                mt = mtE if par == 0 else mtO
                nc.vector.scalar_tensor_tensor(
                    out=sm[:, par, s],
                    in0=sml[0:TH, 16 * par + 8 * hf:16 * par + 8 * hf + 8],
                    scalar=b3bc, in1=mt[:, b0:b0 + 8],
                    op0=OP.add, op1=OP.mult)

        def pool_chunk(sg, loc):  # pool matmuls for items 2*loc, 2*loc+1
            kb = state[("kb", sg)]
            sml = state[("sml", sg)]
            sm = state[("sm", sg)]
            ops = []
            for j in range(2):
                item = 2 * loc + j
                c = 32 + item
                ops.append(lambda item=item, c=c: (
                    nc.tensor.matmul(sml[:, c:c + 1], lhsT=kb[:, item, 0, :],
                                     rhs=sm[:, 0, item:item + 1],
                                     start=True, stop=False),
                    nc.tensor.matmul(sml[:, c:c + 1], lhsT=kb[:, item, 1, :],
                                     rhs=sm[:, 1, item:item + 1],
                                     start=False, stop=True)))
            return ops

        def pool_evac(sg):
            b0 = sg * SG
            sml = state[("sml", sg)]
            nc.vector.tensor_copy(out=poolt_sb[:, b0:b0 + SG],
                                  in_=sml[:, 32:32 + SG])

        # ---- prologue ----
        dma_kb(0, nsplit=4)
        dma_kb(1)
        dma_kb(2)
        dma_kb(3)
        w1q_slab(0)
        mask_prep()
        alloc_sml(0)
        alloc_sml(1)

        # ---- main software-pipelined loop ----
        for P in range(NP + 24):
            t_ops = stage_A(P) if P < NP else []
            f_ops = stage_F(P - 8) if 8 <= P < NP + 8 else []
            b_ops = stage_B(P - 2) if 2 <= P < NP + 2 else []
            d_ops = stage_D(P - 4) if 4 <= P < NP + 4 else []
            p_ops = (pool_chunk((P - 16) // 8, (P - 16) % 8)
                     if 16 <= P < NP + 16 else [])
            for op in t_ops + b_ops + d_ops + f_ops + p_ops:
                op()

            if P % 2 == 1 and P < NP:
                ktevac(P)
            if P % 2 == 1 and 5 <= P < NP + 5:
                relu2((P - 5) // 2)
            if P % 2 == 1 and 3 <= P < NP + 3:
                relu1((P - 3) // 2)
            sg = P // 8
            r = P % 8
            if r == 0 and P < NP:
                dma_kb(sg + 4)
            elif r == 5:
                w1q_slab(sg + 1)
            if 8 <= P < NP + 8 and (P - 8) % 8 == 3:
                do_mask((P - 8) // 8, 0)
            if 8 <= P < NP + 8 and (P - 8) % 8 == 7:
                do_mask((P - 8) // 8, 1)
                if (P - 8) // 8 + 2 < NSG:
                    alloc_sml((P - 8) // 8 + 2)
            if 16 <= P < NP + 16 and (P - 16) % 8 == 7:
                pool_evac((P - 16) // 8)

        # ---- epilogue: transpose pooled back to (batch, E) and store ----
        out_flat = out_d.rearrange("b 1 e -> b e")
        for i in range(0, b_core, 128):
            cnt = min(128, b_core - i)
            ops = sml_ps.tile([128, 128], F32, tag="sml", name="ops")
            nc.tensor.transpose(ops[:cnt, :], poolt_sb[:, i:i + cnt], identf)
            onat = prep.tile([128, E], F32, tag="onat")
            nc.vector.tensor_copy(out=onat[:cnt, :], in_=ops[:cnt, :])
            nc.sync.dma_start(out=out_flat[i:i + cnt, :], in_=onat[:cnt, :])


_NC_CACHE = {}


def _get_nc(b_core=B_CORE):
    if b_core not in _NC_CACHE:
        _NC_CACHE[b_core] = build(b_core)
    return _NC_CACHE[b_core]


def kernel(query, keys, key_masks, W1, b1, W2, b2, W3, b3, _trace=False):
    query = np.ascontiguousarray(query, dtype=np.float32)
    keys = np.ascontiguousarray(keys, dtype=np.float32)
    masks_u8 = np.ascontiguousarray(key_masks).view(np.uint8)
    nc = _get_nc()
    in_maps = []
    for c in range(N_CORES):
        sl = slice(c * B_CORE, (c + 1) * B_CORE)
        in_maps.append({
            "query": query[sl],
            "keys": keys[sl],
            "key_masks": masks_u8[sl],
            "W1": np.asarray(W1, dtype=np.float32),
            "b1": np.asarray(b1, dtype=np.float32),
            "W2": np.asarray(W2, dtype=np.float32),
            "b2": np.asarray(b2, dtype=np.float32),
            "W3": np.asarray(W3, dtype=np.float32),
            "b3": np.asarray(b3, dtype=np.float32),
        })
    res = run_bass_kernel_spmd(nc, in_maps, list(range(N_CORES)), trace=_trace)
    out = np.concatenate([res.results[c]["out"] for c in range(N_CORES)], axis=0)
    if _trace:
        kernel.last_exec_time_ns = res.exec_time_ns
        kernel.last_results = res
    return out.astype(np.float32)


kernel.last_exec_time_ns = None
kernel.last_results = None


# revision 23
# speedup vs baseline: 1.5978x; 1.5907x over previous
"""Trainium2 Bass kernel v4 for AttentionSequencePoolingLayer.

Keys are DMA'd with an inline fp32->bf16 cast on the SWDGE (gpsimd)
path -- 16 DMA engines at ~32ns/KB descriptor vs 10 at 57ns on HWDGE --
and no separate cast op. Per-item fused L1 weights w1q (80 cols) make
layer 1 one matmul per item; the per-item constant C is broadcast into
PSUM by a cheap identity matmul per pair (keeps ACT instruction count
low). relu1/relu2 run per-quad on ACT (800 cols each), kt evacuations
run on DVE as int32 (half the elements), masking is one fused
(score+b3)*mask DVE op, and w1q is built per-sg with two DVE ops.

Sharding: pure data parallel, batch split across 8 cores (256 each).
"""

from contextlib import ExitStack

import numpy as np

import concourse.bass as bass
import concourse.bacc as bacc
import concourse.tile as tile
from concourse import mybir
from concourse.bass_utils import run_bass_kernel_spmd
from concourse.masks import make_identity

B, T, E = 2048, 200, 128
H1, H2 = 80, 40
N_CORES = 8
B_CORE = B // N_CORES   # 256
SG = 16                 # items per supergroup (one keys DMA)
TH = T // 2             # 100 (t-pairs on partitions)
NP = B_CORE // 2        # 128 pairs
NSG = B_CORE // SG      # 16

F32 = mybir.dt.float32
F32R = mybir.dt.float32r
BF16 = mybir.dt.bfloat16
I32 = mybir.dt.int32
U8 = mybir.dt.uint8
AF = mybir.ActivationFunctionType
OP = mybir.AluOpType


def build(b_core=B_CORE):
    nc = bacc.Bacc("TRN2", target_bir_lowering=False, debug=False,
                   num_devices=N_CORES)
    q_d = nc.dram_tensor("query", [b_core, 1, E], F32, kind="ExternalInput")
    k_d = nc.dram_tensor("keys", [b_core, T, E], F32, kind="ExternalInput")
    m_d = nc.dram_tensor("key_masks", [b_core, 1, T], U8, kind="ExternalInput")
    w1_d = nc.dram_tensor("W1", [4 * E, H1], F32, kind="ExternalInput")
    b1_d = nc.dram_tensor("b1", [H1], F32, kind="ExternalInput")
    w2_d = nc.dram_tensor("W2", [H1, H2], F32, kind="ExternalInput")
    b2_d = nc.dram_tensor("b2", [H2], F32, kind="ExternalInput")
    w3_d = nc.dram_tensor("W3", [H2, 1], F32, kind="ExternalInput")
    b3_d = nc.dram_tensor("b3", [1], F32, kind="ExternalInput")
    out_d = nc.dram_tensor("out", [b_core, 1, E], F32, kind="ExternalOutput")

    with tile.TileContext(nc) as tc:
        _body(tc, nc, q_d, k_d, m_d, w1_d, b1_d, w2_d, b2_d, w3_d, b3_d,
              out_d, b_core)
    nc.compile()
    return nc


def _body(tc, nc, q_d, k_d, m_d, w1_d, b1_d, w2_d, b2_d, w3_d, b3_d, out_d,
          b_core):
    ctx = ExitStack()
    with ctx:
        consts = ctx.enter_context(tc.tile_pool(name="consts", bufs=1))
        prep = ctx.enter_context(tc.tile_pool(name="prep", bufs=2))
        ktp_ps = ctx.enter_context(tc.tile_pool(name="ktp_ps", bufs=1,
                                                space="PSUM"))
        h1_ps = ctx.enter_context(tc.tile_pool(name="h1_ps", bufs=1,
                                               space="PSUM"))
        h2_ps = ctx.enter_context(tc.tile_pool(name="h2_ps", bufs=1,
                                               space="PSUM"))
        sml_ps = ctx.enter_context(tc.tile_pool(name="sml_ps", bufs=2,
                                                space="PSUM"))

        identf = consts.tile([128, 128], F32)
        make_identity(nc, identf)
        identb = consts.tile([128, 128], BF16)
        nc.vector.tensor_copy(out=identb, in_=identf)

        # ---- weights ----
        w1s = consts.tile([128, 4, H1], F32)
        nc.sync.dma_start(out=w1s, in_=w1_d.rearrange("(f p) c -> p f c", p=128))
        w1ac = consts.tile([128, H1], F32R)
        nc.vector.tensor_tensor(out=w1ac, in0=w1s[:, 0, :], in1=w1s[:, 2, :],
                                op=OP.add)
        w1db = consts.tile([128, H1], BF16)
        nc.vector.tensor_copy(out=w1db, in_=w1s[:, 3, :])
        w1bcb = consts.tile([128, H1], BF16)
        nc.vector.tensor_tensor(out=w1bcb, in0=w1s[:, 1, :], in1=w1s[:, 2, :],
                                op=OP.subtract)
        w2f = consts.tile([H1, H2], F32)
        nc.sync.dma_start(out=w2f, in_=w2_d.ap())
        w2b = consts.tile([128, H2], BF16)
        nc.vector.memset(w2b, 0.0)
        nc.vector.tensor_copy(out=w2b[0:H1, :], in_=w2f)
        w3f = consts.tile([H2, 1], F32)
        nc.sync.dma_start(out=w3f, in_=w3_d.ap())
        w3p = consts.tile([H2, 1], BF16)
        nc.vector.tensor_copy(out=w3p, in_=w3f)
        b1 = consts.tile([H1, 1], F32)
        nc.sync.dma_start(
            out=b1, in_=bass.AP(tensor=b1_d.ap().tensor, offset=0,
                                ap=[[1, H1], [1, 1]]))
        b2 = consts.tile([H2, 1], F32)
        nc.sync.dma_start(
            out=b2, in_=bass.AP(tensor=b2_d.ap().tensor, offset=0,
                                ap=[[1, H2], [1, 1]]))
        b3bc = consts.tile([TH, 1], F32)
        nc.sync.dma_start(
            out=b3bc, in_=bass.AP(tensor=b3_d.ap().tensor, offset=0,
                                  ap=[[0, TH], [1, 1]]))

        # ---- Q.T (E on partitions, batch on free) ----
        qt = consts.tile([128, b_core], F32R)
        q_flat = q_d.rearrange("b 1 e -> b e")
        for i in range(0, b_core, 128):
            cnt = min(128, b_core - i)
            qnat = prep.tile([128, E], F32, tag="qnat")
            nc.sync.dma_start(out=qnat[:cnt, :], in_=q_flat[i:i + cnt, :])
            qps = sml_ps.tile([128, 128], F32, tag="sml", name="qps")
            nc.tensor.transpose(qps[:, :cnt], qnat[:cnt, :], identf[:cnt, :cnt])
            nc.vector.tensor_copy(out=qt[:, i:i + cnt], in_=qps[:, :cnt])

        # ---- C = (W1a+W1c).T @ Q.T + b1 -> csbb [128, b_core] bf16 ----
        cps = sml_ps.tile([H1, b_core], F32, tag="sml", name="cps")
        nc.tensor.matmul(cps, lhsT=w1ac, rhs=qt, start=True, stop=True)
        csbb = consts.tile([128, b_core], BF16)
        nc.vector.memset(csbb, 0.0)
        nc.scalar.activation(out=csbb[0:H1, :], in_=cps, func=AF.Identity,
                             bias=b1)

        # masks prepared later (mask_prep), after prologue casts
        mtE = consts.tile([TH, b_core], F32)
        mtO = consts.tile([TH, b_core], F32)

        def mask_prep():
            m_flat = m_d.rearrange("b 1 t -> b t")
            for i in range(0, b_core, 128):
                cnt = min(128, b_core - i)
                mu8 = prep.tile([128, T], U8, tag="mu8")
                nc.sync.dma_start(out=mu8[:cnt, :], in_=m_flat[i:i + cnt, :])
                mf = prep.tile([128, T], F32, tag="mf")
                nc.vector.tensor_copy(out=mf[:cnt, :], in_=mu8[:cnt, :])
                mfv = mf.rearrange("p (t two) -> p two t", two=2)
                for par, mt in ((0, mtE), (1, mtO)):
                    mps = sml_ps.tile([128, 128], F32, tag="sml", name="mps")
                    nc.tensor.transpose(mps[:TH, :cnt], mfv[:cnt, par, :],
                                        identf[:cnt, :cnt])
                    nc.vector.tensor_copy(out=mt[:, i:i + cnt],
                                          in_=mps[:TH, :cnt])

        qtb = consts.tile([128, b_core], BF16)
        nc.vector.tensor_copy(out=qtb, in_=qt.bitcast(F32))

        # pooled output, transposed: (E, batch)
        poolt_sb = consts.tile([128, b_core], F32)

        # ---- main pipeline pools ----
        kbp = ctx.enter_context(tc.tile_pool(name="kbp", bufs=7))
        w1qp = ctx.enter_context(tc.tile_pool(name="w1qp", bufs=3))
        ktq = ctx.enter_context(tc.tile_pool(name="ktq", bufs=3))
        h1qp = ctx.enter_context(tc.tile_pool(name="h1qp", bufs=3))
        h2qp = ctx.enter_context(tc.tile_pool(name="h2qp", bufs=3))
        smp = ctx.enter_context(tc.tile_pool(name="smp", bufs=2))

        # psum tiles: bank-safe layouts (no matmul write crosses a 2KB bank)
        ktpA = ktp_ps.tile([128, 2, 2, TH * 2], BF16, name="ktpA")  # 1 bank
        ktpB = ktp_ps.tile([128, 2, 2, TH * 2], BF16, name="ktpB")  # 1 bank
        h1ps = h1_ps.tile([128, 2, 512], F32, name="h1ps")  # 2 banks, padded
        h2ps = h2_ps.tile([H2, 2, 512], F32, name="h2ps")   # 2 banks, padded

        kE = T * E  # element stride between items in keys dram
        k_t = k_d.ap().tensor

        state = {}

        def dma_kb(sg, nsplit=1):
            if not (0 <= sg < NSG):
                return
            kb = kbp.tile([TH, SG, 2, E], BF16, tag="kb", name="kb")
            kbv = kb.rearrange("p s a e -> p s (a e)")
            step = SG // nsplit
            for c in range(nsplit):
                nc.gpsimd.dma_start(
                    out=kbv[:, c * step:(c + 1) * step, :],
                    in_=bass.AP(tensor=k_t,
                                offset=(sg * SG + c * step) * kE,
                                ap=[[2 * E, TH], [kE, step], [1, 2 * E]]))
            state[("kb", sg)] = kb

        def w1q_slab(sg):
            # w1q[:, i, 0:80] = (W1d * q_i) + (W1b - W1c); cols 80:128 zero
            # so the L1 lhsT is 128-wide (FWL). Two DVE ops + a pad memset.
            if not (0 <= sg < NSG):
                return
            b0 = sg * SG
            tmp = w1qp.tile([128, SG, H1], BF16, tag="w1qtmp", name="tmp",
                            bufs=2)
            w1d3 = bass.AP(tensor=w1db.tensor, offset=w1db.offset,
                           ap=[w1db.ap[0], [0, SG], w1db.ap[1]])
            qsl = qtb[:, b0:b0 + SG]
            qt3 = bass.AP(tensor=qsl.tensor, offset=qsl.offset,
                          ap=[qsl.ap[0], qsl.ap[1], [0, H1]])
            nc.vector.tensor_tensor(out=tmp, in0=w1d3, in1=qt3, op=OP.mult)
            w1q = w1qp.tile([128, SG, 128], BF16, tag="w1q", name="w1q")
            if sg < 3:
                # pads zeroed once per physical buffer (bufs=3, rotation)
                nc.vector.memset(w1q[:, :, H1:128], 0.0)
            w1bc3 = bass.AP(tensor=w1bcb.tensor, offset=w1bcb.offset,
                            ap=[w1bcb.ap[0], [0, SG], w1bcb.ap[1]])
            nc.vector.tensor_tensor(out=w1q[:, :, 0:H1], in0=tmp, in1=w1bc3,
                                    op=OP.add)
            state[("w1q", sg)] = w1q

        def stage_A(P):  # transposes of pair P -> ktp quad tile, half P%2
            sg, loc = P // 8, P % 8
            kb = state[("kb", sg)]
            ktp = ktpA if (P // 2) % 2 == 0 else ktpB
            half = P % 2
            ops = []
            for j in range(2):
                item = 2 * loc + j
                for par in range(2):
                    ops.append(lambda j=j, item=item, par=par: nc.tensor.transpose(
                        ktp[:, half, j, par * TH:(par + 1) * TH],
                        kb[:, item, par, :], identb[:TH, :TH]))
            return ops

        def ktevac(P):  # evacuate one quad's ktp tile -> kt sbuf (DVE, i32)
            q = (P - 1) // 2
            ktp = ktpA if q % 2 == 0 else ktpB
            kt = ktq.tile([128, 2, 2, 2 * TH], BF16, tag="kt", name="kt")
            ktv = kt.rearrange("p a b t -> p (a b t)").bitcast(I32)
            ktpv = ktp.rearrange("p a b t -> p (a b t)").bitcast(I32)
            nc.vector.tensor_copy(out=ktv, in_=ktpv)
            state[("kt", q)] = kt

        def stage_B(P):  # L1 for pair P: C broadcast + matmul per item
            q, half = P // 2, P % 2
            kt = state[("kt", q)]
            sg = P // 8
            w1q = state[("w1q", sg)]
            csl = csbb[:, 2 * P:2 * P + 2]
            crhs = bass.AP(tensor=csl.tensor, offset=csl.offset,
                           ap=[csl.ap[0], csl.ap[1], [0, 200]])
            ops = [lambda: nc.tensor.matmul(
                h1ps[:, half, 0:400], lhsT=identb, rhs=crhs,
                start=True, stop=False)]
            for j in range(2):
                item = (P % 8) * 2 + j
                ops.append(lambda j=j, item=item: nc.tensor.matmul(
                    h1ps[:, half, j * 200:(j + 1) * 200],
                    lhsT=w1q[:, item, :], rhs=kt[:, half, j, :],
                    start=False, stop=(j == 1)))
            return ops

        def relu1(q):  # quad q: h1 = relu(h1ps) on ACT, used cols only
            h1q = h1qp.tile([128, 2, 512], BF16, tag="h1q", name="h1q")
            nc.scalar.activation(out=h1q[:, :, 0:400], in_=h1ps[:, :, 0:400],
                                 func=AF.Relu)
            state[("h1q", q)] = h1q

        def stage_D(P):  # L2 for pair P (K=80)
            half = P % 2
            h1q = state[("h1q", P // 2)]
            return [lambda: nc.tensor.matmul(
                h2ps[:, half, 0:400], lhsT=w2b, rhs=h1q[:, half, 0:400],
                start=True, stop=True)]

        def relu2(q):  # quad q: h2 = relu(h2ps + b2) on ACT
            h2q = h2qp.tile([H2, 2, 512], BF16, tag="h2q", name="h2q")
            nc.scalar.activation(out=h2q[:, :, 0:400], in_=h2ps[:, :, 0:400],
                                 func=AF.Relu, bias=b2)
            state[("h2q", q)] = h2q

        def stage_F(P):  # score minis for pair P
            q, half = P // 2, P % 2
            h2q = state[("h2q", q)]
            sml = state[("sml", P // 8)]
            ops = []
            for j in range(2):
                li = (P % 8) * 2 + j
                for par in range(2):
                    ops.append(lambda j=j, li=li, par=par: nc.tensor.matmul(
                        sml[:, 16 * par + li:16 * par + li + 1],
                        lhsT=h2q[:, half, j * 200 + par * TH:
                                 j * 200 + par * TH + 128],
                        rhs=w3p, start=True, stop=True))
            return ops

        def alloc_sml(sg):
            sml = sml_ps.tile([128, 128], F32, tag="sml", name="sml")
            state[("sml", sg)] = sml

        def do_mask(sg, hf):  # sm = (score + b3) * mask, fused per parity
            b0 = sg * SG + 8 * hf
            sml = state[("sml", sg)]
            if hf == 0:
                sm = smp.tile([TH, 2, SG], BF16, tag="sm", name="sm")
                state[("sm", sg)] = sm
            sm = state[("sm", sg)]
            s = slice(8 * hf, 8 * hf + 8)
            for par in range(2):
                mt = mtE if par == 0 else mtO
                nc.vector.scalar_tensor_tensor(
                    out=sm[:, par, s],
                    in0=sml[0:TH, 16 * par + 8 * hf:16 * par + 8 * hf + 8],
                    scalar=b3bc, in1=mt[:, b0:b0 + 8],
                    op0=OP.add, op1=OP.mult)

        def pool_chunk(sg, loc):  # pool matmuls for items 2*loc, 2*loc+1
            kb = state[("kb", sg)]
            sml = state[("sml", sg)]
            sm = state[("sm", sg)]
            ops = []
            for j in range(2):
                item = 2 * loc + j
                c = 32 + item
                ops.append(lambda item=item, c=c: (
                    nc.tensor.matmul(sml[:, c:c + 1], lhsT=kb[:, item, 0, :],
                                     rhs=sm[:, 0, item:item + 1],
                                     start=True, stop=False),
                    nc.tensor.matmul(sml[:, c:c + 1], lhsT=kb[:, item, 1, :],
                                     rhs=sm[:, 1, item:item + 1],
                                     start=False, stop=True)))
            return ops

        def pool_evac(sg):
            b0 = sg * SG
            sml = state[("sml", sg)]
            nc.vector.tensor_copy(out=poolt_sb[:, b0:b0 + SG],
                                  in_=sml[:, 32:32 + SG])

        # ---- prologue ----
        dma_kb(0, nsplit=4)
        dma_kb(1)
        dma_kb(2)
        dma_kb(3)
        w1q_slab(0)
        mask_prep()
        alloc_sml(0)
        alloc_sml(1)

        # ---- main software-pipelined loop ----
        for P in range(NP + 24):
            t_ops = stage_A(P) if P < NP else []
            f_ops = stage_F(P - 8) if 8 <= P < NP + 8 else []
            b_ops = stage_B(P - 2) if 2 <= P < NP + 2 else []
            d_ops = stage_D(P - 4) if 4 <= P < NP + 4 else []
            p_ops = (pool_chunk((P - 16) // 8, (P - 16) % 8)
                     if 16 <= P < NP + 16 else [])
            for op in t_ops + b_ops + d_ops + f_ops + p_ops:
                op()

            if P % 2 == 1 and P < NP:
                ktevac(P)
            if P % 2 == 1 and 5 <= P < NP + 5:
                relu2((P - 5) // 2)
            if P % 2 == 1 and 3 <= P < NP + 3:
                relu1((P - 3) // 2)
            sg = P // 8
            r = P % 8
            if r == 0 and P < NP:
                dma_kb(sg + 4)
            elif r == 5:
                w1q_slab(sg + 1)
            if 8 <= P < NP + 8 and (P - 8) % 8 == 3:
                do_mask((P - 8) // 8, 0)
            if 8 <= P < NP + 8 and (P - 8) % 8 == 7:
                do_mask((P - 8) // 8, 1)
                if (P - 8) // 8 + 2 < NSG:
                    alloc_sml((P - 8) // 8 + 2)
            if 16 <= P < NP + 16 and (P - 16) % 8 == 7:
                pool_evac((P - 16) // 8)

        # ---- epilogue: transpose pooled back to (batch, E) and store ----
        out_flat = out_d.rearrange("b 1 e -> b e")
        for i in range(0, b_core, 128):
            cnt = min(128, b_core - i)
            ops = sml_ps.tile([128, 128], F32, tag="sml", name="ops")
            nc.tensor.transpose(ops[:cnt, :], poolt_sb[:, i:i + cnt], identf)
            onat = prep.tile([128, E], F32, tag="onat")
            nc.vector.tensor_copy(out=onat[:cnt, :], in_=ops[:cnt, :])
            nc.sync.dma_start(out=out_flat[i:i + cnt, :], in_=onat[:cnt, :])


_NC_CACHE = {}


def _get_nc(b_core=B_CORE):
    if b_core not in _NC_CACHE:
        _NC_CACHE[b_core] = build(b_core)
    return _NC_CACHE[b_core]


def kernel(query, keys, key_masks, W1, b1, W2, b2, W3, b3, _trace=False):
    query = np.ascontiguousarray(query, dtype=np.float32)
    keys = np.ascontiguousarray(keys, dtype=np.float32)
    masks_u8 = np.ascontiguousarray(key_masks).view(np.uint8)
    nc = _get_nc()
    in_maps = []
    for c in range(N_CORES):
        sl = slice(c * B_CORE, (c + 1) * B_CORE)
        in_maps.append({
            "query": query[sl],
            "keys": keys[sl],
            "key_masks": masks_u8[sl],
            "W1": np.asarray(W1, dtype=np.float32),
            "b1": np.asarray(b1, dtype=np.float32),
            "W2": np.asarray(W2, dtype=np.float32),
            "b2": np.asarray(b2, dtype=np.float32),
            "W3": np.asarray(W3, dtype=np.float32),
            "b3": np.asarray(b3, dtype=np.float32),
        })
    res = run_bass_kernel_spmd(nc, in_maps, list(range(N_CORES)), trace=_trace)
    out = np.concatenate([res.results[c]["out"] for c in range(N_CORES)], axis=0)
    if _trace:
        kernel.last_exec_time_ns = res.exec_time_ns
        kernel.last_results = res
    return out.astype(np.float32)


kernel.last_exec_time_ns = None
kernel.last_results = None
